# revision 1
# baseline (speedup 1.0000x reference)
"""CRF decoder loss kernel for Trainium2 (Bass/Tile), 8-core data parallel.

Algorithm notes
---------------
The CRF forward algorithm is computed in the "hot" (exp) domain:
    u_{t+1}[j,b] = el_t[j,b] * sum_i exp(T[j,i]) * u_t[i,b]
with el_t = exp(logit_t + bias - C0).  Each step is one PE matmul
(stationary exp(T)^T, 52x52) + one DVE elementwise multiply (52x16).
A constant e^{-C0} per step keeps magnitudes near 1; every R steps a
data-dependent rescale (divide by the state-mass sum, accumulate log)
bounds fp32 range; its multiply is applied DEF steps late so it never
stalls the serial chain.  State row 51 (END) has zero incoming weights
in exp(T)^T (transition from END is -100), so it is repurposed to carry
the "end-dot" sum_i exp(T[END,i]) u_t[i] forward one step -- giving the
norm-score numerator for every prefix length without extra copies.
Per-sequence lengths select the right prefix via host-built one-hot
matrices (pure index preprocessing of `lens`/`labels`).

Emission logits are produced chunk-by-chunk with float32r matmuls and
the chunk-(ch+1) matmuls are interleaved between scan steps of chunk ch
so the in-order PE queue never head-of-line blocks the scan chain.

Gold score = sum(onehot*mask (.) logits) + sum(paircount (.) T) + label
counts (.) bias, all reduced on device.

Sharding: pure data parallel over batch (16 sequences/core); final
scalar partial losses summed on host.
"""

import numpy as np
from contextlib import ExitStack

import concourse.bass as bass
import concourse.tile as tile
from concourse import bacc
from concourse import mybir
from concourse.bass_utils import run_bass_kernel_spmd

F32 = mybir.dt.float32
F32R = mybir.dt.float32r
AF = mybir.ActivationFunctionType
ALU = mybir.AluOpType

B, S, D = 128, 512, 1024
L = 50            # real labels
NL = L + 2        # + START, END
START, END = 50, 51
NCORES = 8
BL = B // NCORES  # 16 sequences per core
TCH = 32          # timesteps per emission chunk
NCHUNK = S // TCH
KD = D // 128     # contraction chunks for emission matmul
R = 16            # rescale period (steps)
DEF = 3           # rescale apply deferral (steps)
NEV = S // R      # rescale event slots (last one unused)
C0 = 7.5          # constant per-step log damping folded into emission bias


def build_program():
    nc = bacc.Bacc("TRN2", target_bir_lowering=False, debug=False,
                   num_devices=NCORES)

    xT_d = nc.dram_tensor("xT", [KD, 128, S * BL], F32R, kind="ExternalInput")
    WT_d = nc.dram_tensor("WT", [KD, 128, L], F32R, kind="ExternalInput")
    bias_d = nc.dram_tensor("bias", [L, 1], F32, kind="ExternalInput")
    TT_d = nc.dram_tensor("TT", [NL, NL], F32, kind="ExternalInput")
    T_d = nc.dram_tensor("Tm", [NL, NL], F32, kind="ExternalInput")
    OH_d = nc.dram_tensor("OH", [L, S * BL], F32, kind="ExternalInput")
    CNT_d = nc.dram_tensor("CNT", [NL, NL], F32, kind="ExternalInput")
    CNTL_d = nc.dram_tensor("CNTL", [L, 1], F32, kind="ExternalInput")
    SELEND_d = nc.dram_tensor("SELEND", [128, 4 * BL], F32, kind="ExternalInput")
    SELSC_d = nc.dram_tensor("SELSC", [NEV + 1, BL], F32, kind="ExternalInput")
    SCC_d = nc.dram_tensor("SCC", [1, BL], F32, kind="ExternalInput")
    ELINIT_d = nc.dram_tensor("ELINIT", [2, S * BL], F32, kind="ExternalInput")
    UINIT_d = nc.dram_tensor("UINIT", [NL, BL], F32, kind="ExternalInput")
    loss_d = nc.dram_tensor("loss", [1, 1], F32, kind="ExternalOutput")
    dbg_d = nc.dram_tensor("dbg", [1, BL], F32, kind="ExternalOutput")

    with tile.TileContext(nc) as tc, ExitStack() as ctx:
        consts = ctx.enter_context(tc.tile_pool(name="consts", bufs=1))
        xpool = ctx.enter_context(tc.tile_pool(name="xpool", bufs=3))
        ohpool = ctx.enter_context(tc.tile_pool(name="ohpool", bufs=3))
        smalls = ctx.enter_context(tc.tile_pool(name="smalls", bufs=2))
        lgp = ctx.enter_context(tc.tile_pool(name="lgp", bufs=2, space="PSUM"))
        pp = ctx.enter_context(tc.tile_pool(name="pp", bufs=3, space="PSUM"))
        miscp = ctx.enter_context(tc.tile_pool(name="miscp", bufs=1, space="PSUM"))

        # ---------------- constants ----------------
        ttile = consts.tile([NL, NL], F32, name="ttile")
        nc.sync.dma_start(out=ttile[:, :], in_=TT_d.ap()[:, :])
        stat = consts.tile([NL, NL], F32, name="stat")  # stat[i,j] = exp(T[j,i])
        nc.scalar.activation(out=stat[:, :], in_=ttile[:, :], func=AF.Exp)

        wt = consts.tile([128, KD * L], F32R, name="wt")
        for k in range(KD):
            nc.sync.dma_start(out=wt[:, k * L:(k + 1) * L], in_=WT_d.ap()[k, :, :])
        braw = consts.tile([L, 1], F32, name="braw")
        nc.sync.dma_start(out=braw[:, :], in_=bias_d.ap()[:, :])
        btile = consts.tile([L, 1], F32, name="btile")
        nc.vector.tensor_scalar_add(btile[:, :], braw[:, :], -C0)

        ones = consts.tile([128, 1], F32, name="ones")
        nc.vector.memset(ones[:, :], 1.0)
        ones_r = consts.tile([1, NL], F32, name="ones_r")
        nc.vector.memset(ones_r[:, :], 1.0)

        traw = consts.tile([NL, NL], F32, name="traw")
        nc.sync.dma_start(out=traw[:, :], in_=T_d.ap()[:, :])
        cnt = consts.tile([NL, NL], F32, name="cnt")
        nc.sync.dma_start(out=cnt[:, :], in_=CNT_d.ap()[:, :])
        cntl = consts.tile([L, 1], F32, name="cntl")
        nc.sync.dma_start(out=cntl[:, :], in_=CNTL_d.ap()[:, :])
        selend = consts.tile([128, 4 * BL], F32, name="selend")
        nc.sync.dma_start(out=selend[:, :], in_=SELEND_d.ap()[:, :])
        selsc = consts.tile([NEV + 1, BL], F32, name="selsc")
        nc.sync.dma_start(out=selsc[:, :], in_=SELSC_d.ap()[:, :])

        # ---------------- big state buffers ----------------
        el_buf = consts.tile([NL, S * BL], F32, name="el_buf")
        u_buf = consts.tile([NL, (S + 2) * BL], F32, name="u_buf")
        scale_row = consts.tile([1, NEV * BL], F32, name="scale_row")
        uacc = consts.tile([L, NCHUNK], F32, name="uacc")
        scratch = consts.tile([NL, TCH * BL], F32, name="scratch")

        nc.vector.memset(scale_row[:, :], 0.0)
        nc.sync.dma_start(out=el_buf[START:START + 2, :], in_=ELINIT_d.ap()[:, :])
        nc.sync.dma_start(out=u_buf[:, 0:BL], in_=UINIT_d.ap()[:, :])

        # ---------------- emission helpers ----------------
        xt_tiles = {}
        oh_tiles = {}
        lg_tiles = {}

        def issue_dma(ch):
            xt = xpool.tile([128, KD * TCH * BL], F32R, name="xt", tag="xt")
            for k in range(KD):
                nc.sync.dma_start(out=xt[:, k * TCH * BL:(k + 1) * TCH * BL],
                                  in_=xT_d.ap()[k, :, ch * TCH * BL:(ch + 1) * TCH * BL])
            oh = ohpool.tile([L, TCH * BL], F32, name="oh", tag="oh")
            nc.sync.dma_start(out=oh[:, :],
                              in_=OH_d.ap()[:, ch * TCH * BL:(ch + 1) * TCH * BL])
            xt_tiles[ch] = xt
            oh_tiles[ch] = oh

        def em_mm(ch, k):
            if k == 0:
                lg_tiles[ch] = lgp.tile([L, TCH * BL], F32, name="lg", tag="lg")
            lg = lg_tiles[ch]
            xt = xt_tiles[ch]
            nc.tensor.matmul(
                lg[:, :],
                lhsT=wt[:, k * L:(k + 1) * L],
                rhs=xt[:, k * TCH * BL:(k + 1) * TCH * BL],
                start=(k == 0), stop=(k == KD - 1))

        def em_exp(ch):
            csl = slice(ch * TCH * BL, (ch + 1) * TCH * BL)
            nc.scalar.activation(out=el_buf[0:L, csl], in_=lg_tiles[ch][:, :],
                                 func=AF.Exp, bias=btile[:, 0:1], scale=1.0)

        def em_unary_mul(ch):
            nc.vector.tensor_mul(scratch[0:L, :], lg_tiles[ch][:, :],
                                 oh_tiles[ch][:, :])

        def em_unary_red(ch):
            nc.vector.tensor_reduce(out=uacc[:, ch:ch + 1], in_=scratch[0:L, :],
                                    axis=mybir.AxisListType.X, op=ALU.add)

        # chunk 0 emission upfront
        issue_dma(0)
        issue_dma(1)
        for k in range(KD):
            em_mm(0, k)
        em_exp(0)
        em_unary_mul(0)
        em_unary_red(0)

        # pending rescale state: (apply_step, pb_tile)
        pend_apply = {}

        # ---------------- scan with interleaved emission ----------------
        for ch in range(NCHUNK):
            if ch + 2 < NCHUNK:
                issue_dma(ch + 2)
            for tl in range(TCH):
                t = ch * TCH + tl
                p = pp.tile([NL, BL], F32, name="p", tag="p")
                nc.tensor.matmul(p[:, :], lhsT=stat[:, :],
                                 rhs=u_buf[:, t * BL:(t + 1) * BL],
                                 start=True, stop=True)
                nc.vector.tensor_mul(u_buf[:, (t + 1) * BL:(t + 2) * BL],
                                     p[:, :], el_buf[:, t * BL:(t + 1) * BL])

                # deferred rescale apply
                if t in pend_apply:
                    pb = pend_apply.pop(t)
                    nc.vector.tensor_mul(
                        u_buf[0:START, (t + 1) * BL:(t + 2) * BL],
                        u_buf[0:START, (t + 1) * BL:(t + 2) * BL],
                        pb[0:START, :])

                # rescale event: record log-sum and queue deferred apply
                if t % R == R - 1 and t + 1 + DEF <= S:
                    kev = t // R
                    ps = miscp.tile([1, BL], F32, name="ps", tag="m1")
                    nc.tensor.matmul(ps[:, :], lhsT=ones[0:L, :],
                                     rhs=u_buf[0:L, (t + 1) * BL:(t + 2) * BL],
                                     start=True, stop=True)
                    nc.scalar.activation(
                        out=scale_row[:, kev * BL:(kev + 1) * BL],
                        in_=ps[:, :], func=AF.Ln)
                    rec = smalls.tile([1, BL], F32, name="rec", tag="rec")
                    nc.vector.reciprocal(rec[:, :], ps[:, :])
                    pb = miscp.tile([NL, BL], F32, name="pb", tag="m2")
                    nc.tensor.matmul(pb[:, :], lhsT=ones_r[:, :], rhs=rec[:, :],
                                     start=True, stop=True)
                    pend_apply[t + DEF] = pb

                # interleaved emission for chunk ch+1
                if ch + 1 < NCHUNK:
                    if tl % 4 == 0:
                        em_mm(ch + 1, tl // 4)
                    elif tl == 29:
                        em_exp(ch + 1)
                    elif tl == 30:
                        em_unary_mul(ch + 1)
                    elif tl == 31:
                        em_unary_red(ch + 1)

        # final end-dot for full-length sequences (prefix L = S)
        pf = pp.tile([NL, BL], F32, name="pf", tag="p")
        nc.tensor.matmul(pf[:, :], lhsT=stat[:, :],
                         rhs=u_buf[:, S * BL:(S + 1) * BL], start=True, stop=True)
        # copy must start at a 32-aligned partition; rows 32..50 of this
        # slice are never read, only row END matters.
        nc.scalar.copy(u_buf[32:NL, (S + 1) * BL:(S + 2) * BL],
                       pf[32:NL, :])

        # ---------------- norm score selection ----------------
        endbuf = consts.tile([128, 4 * BL], F32, name="endbuf")
        for blk in range(4):
            src = u_buf[END:END + 1,
                        (blk * 128 + 2) * BL:(blk * 128 + 130) * BL]
            nc.sync.dma_start(
                out=endbuf[:, blk * BL:(blk + 1) * BL],
                in_=src.rearrange("p (q b) -> p q b", q=128, b=BL))
        nc.vector.tensor_scalar_max(endbuf[:, :], endbuf[:, :], 1e-38)
        endlog = consts.tile([128, 4 * BL], F32, name="endlog")
        nc.scalar.activation(out=endlog[:, :], in_=endbuf[:, :], func=AF.Ln)
        nc.vector.tensor_mul(endlog[:, :], endlog[:, :], selend[:, :])
        esum = consts.tile([128, BL], F32, name="esum")
        nc.vector.tensor_reduce(
            out=esum[:, :],
            in_=endlog.rearrange("p (blk b) -> p b blk", blk=4, b=BL),
            axis=mybir.AxisListType.X, op=ALU.add)

        scsel = consts.tile([NEV + 1, BL], F32, name="scsel")
        nc.sync.dma_start(out=scsel[0:NEV, :],
                          in_=scale_row.rearrange("p (k b) -> p k b", k=NEV, b=BL))
        nc.sync.dma_start(out=scsel[NEV:NEV + 1, :], in_=SCC_d.ap()[:, :])
        nc.vector.tensor_mul(scsel[:, :], scsel[:, :], selsc[:, :])

        nacc = miscp.tile([1, BL], F32, name="nacc", tag="m1")
        nc.tensor.matmul(nacc[:, :], lhsT=ones[:, :], rhs=esum[:, :],
                         start=True, stop=False)
        nc.tensor.matmul(nacc[:, :], lhsT=ones[0:NEV + 1, :], rhs=scsel[:, :],
                         start=False, stop=True)

        # ---------------- gold score ----------------
        gt1 = consts.tile([NL, 1], F32, name="gt1")
        nc.vector.tensor_mul(scratch[0:NL, 0:NL], traw[:, :], cnt[:, :])
        nc.vector.tensor_reduce(out=gt1[:, :], in_=scratch[0:NL, 0:NL],
                                axis=mybir.AxisListType.X, op=ALU.add)
        gt2 = consts.tile([L, 1], F32, name="gt2")
        nc.vector.tensor_mul(gt2[:, :], braw[:, :], cntl[:, :])
        ur = consts.tile([L, 1], F32, name="ur")
        nc.vector.tensor_reduce(out=ur[:, :], in_=uacc[:, :],
                                axis=mybir.AxisListType.X, op=ALU.add)
        gacc = miscp.tile([1, 1], F32, name="gacc", tag="m2")
        nc.tensor.matmul(gacc[:, :], lhsT=ones[0:NL, :], rhs=gt1[:, :],
                         start=True, stop=False)
        nc.tensor.matmul(gacc[:, :], lhsT=ones[0:L, :], rhs=gt2[:, :],
                         start=False, stop=False)
        nc.tensor.matmul(gacc[:, :], lhsT=ones[0:L, :], rhs=ur[:, :],
                         start=False, stop=True)

        # loss = sum_b norm - gold
        nr = smalls.tile([1, 1], F32, name="nr", tag="nr")
        nc.vector.tensor_reduce(out=nr[:, :], in_=nacc[:, :],
                                axis=mybir.AxisListType.X, op=ALU.add)
        lt = smalls.tile([1, 1], F32, name="lt", tag="lt")
        nc.vector.tensor_sub(lt[:, :], nr[:, :], gacc[:, :])
        dbgt = smalls.tile([1, BL], F32, name="dbgt", tag="dbgt")
        nc.scalar.copy(dbgt[:, :], nacc[:, :])
        nc.sync.dma_start(out=loss_d.ap()[:, :], in_=lt[:, :])
        nc.sync.dma_start(out=dbg_d.ap()[:, :], in_=dbgt[:, :])

    nc.compile()
    return nc


def prep_inputs(inputs, W, b, transition, lens, labels):
    """Host-side sharding + index preprocessing. Returns per-core input maps."""
    x = np.ascontiguousarray(np.asarray(inputs, dtype=np.float32))
    W = np.asarray(W, dtype=np.float32)
    b = np.asarray(b, dtype=np.float32)
    T = np.asarray(transition, dtype=np.float32)
    lens = np.asarray(lens).astype(np.int64)
    labels = np.asarray(labels).astype(np.int64)

    WT = np.ascontiguousarray(W.T).reshape(KD, 128, L)
    TT = np.ascontiguousarray(T.T)
    bias = b.reshape(L, 1)

    # (B,S,D) -> (D,S,B) once, then per-core contiguous slices
    xt_all = np.ascontiguousarray(np.transpose(x, (2, 1, 0)))  # (D, S, B)

    in_maps = []
    for c in range(NCORES):
        bs = slice(c * BL, (c + 1) * BL)
        lens_c = lens[bs]
        labels_c = labels[bs]

        xT = np.ascontiguousarray(xt_all[:, :, bs]).reshape(KD, 128, S * BL)

        mask = np.arange(S)[:, None] < lens_c[None, :]        # (S, BL)
        lab_t = labels_c.T                                     # (S, BL)
        OH = (lab_t[None, :, :] == np.arange(L)[:, None, None]) & mask[None]
        OH = np.ascontiguousarray(OH.astype(np.float32).reshape(L, S * BL))

        # pair counts following the reference labels_ext construction
        ext = np.full((BL, S + 2), END, dtype=np.int64)
        ext[:, 0] = START
        ext[:, 1:S + 1] = labels_c
        valid = np.arange(S + 2)[None, :] < (lens_c + 1)[:, None]
        ext = np.where(valid, ext, END)
        CNT = np.zeros((NL, NL), dtype=np.float32)
        pmask = np.arange(S + 1)[None, :] < (lens_c + 1)[:, None]
        to_ = ext[:, 1:][pmask]
        fr_ = ext[:, :-1][pmask]
        np.add.at(CNT, (to_, fr_), 1.0)

        CNTL = np.zeros((L,), dtype=np.float32)
        msk = np.arange(S)[None, :] < lens_c[:, None]
        np.add.at(CNTL, labels_c[msk], 1.0)
        CNTL = CNTL.reshape(L, 1)

        SELEND = np.zeros((128, 4 * BL), dtype=np.float32)
        q = lens_c - 1  # 0..511
        SELEND[q % 128, (q // 128) * BL + np.arange(BL)] = 1.0

        # event k (at step 16k+15) is applied to u slice 16k+16+DEF,
        # so it affects end-dots for prefix lengths >= 16k+16+DEF.
        SELSC = np.zeros((NEV + 1, BL), dtype=np.float32)
        for k in range(NEV):
            if R * k + R - 1 + 1 + DEF <= S:
                SELSC[k, :] = (lens_c >= (R * k + R + DEF)).astype(np.float32)
        SELSC[NEV, :] = 1.0
        SCC = (C0 * lens_c.astype(np.float32)).reshape(1, BL)
        ELINIT = np.zeros((2, S * BL), dtype=np.float32)
        ELINIT[1, :] = 1.0
        UINIT = np.zeros((NL, BL), dtype=np.float32)
        UINIT[START, :] = 1.0

        in_maps.append({
            "xT": xT, "WT": WT, "bias": bias, "TT": TT, "Tm": T,
            "OH": OH, "CNT": CNT, "CNTL": CNTL,
            "SELEND": SELEND, "SELSC": SELSC, "SCC": SCC,
            "ELINIT": ELINIT, "UINIT": UINIT,
        })
    return in_maps


_NC_CACHE = []


def kernel(inputs, W, b, transition, lens, labels, _trace=False, _tmpdir=None):
    in_maps = prep_inputs(inputs, W, b, transition, lens, labels)
    if not _NC_CACHE:
        _NC_CACHE.append(build_program())
    nc = _NC_CACHE[0]
    res = run_bass_kernel_spmd(nc, in_maps, list(range(NCORES)),
                               trace=_trace, tmpdir=_tmpdir)
    total = np.float64(0.0)
    for r in res.results:
        total += np.float64(r["loss"][0, 0])
    out = np.float32(total)
    if _trace:
        return out, res
    return out

